# revision 58
# baseline (speedup 1.0000x reference)
"""Trainium2 Bass kernel for nn_MultiHeadAttention_81913616270105.

Module: pre-LN -> QKV linear -> plain-reshape head split -> softmax(QK^T)/sqrt(D)
        -> attn @ V -> out proj -> +residual.   B=2, S=2048, D=1024, H=8.

Key structural fact: the reference splits heads with a plain reshape
(B,S,D)->(B,H,S,DH), so head h of batch b covers token rows 256h..256h+256
of that batch, viewed as 2048 "subtokens" of width 128 (subtoken a = t*8+c
maps to token t, feature block c). The layer is therefore fully row-local:
16 (b,h) blocks of 256 token rows -> 2 blocks per core across 8 cores,
no collectives.

Softmax rows are independent and the k-reduction is order-free, so inside a
block we enumerate q and k subtokens in plain (c, t) memory order rather than
the reference's (t, c) order — every SBUF access in the attention pipeline is
then contiguous; only the final out-projection slices map back to token rows.

Per-core pipeline (512 token rows):
  LN (fp32, bn_stats) -> PE-transpose xhat -> QKV projections (bf16 matmuls,
  W^T stationary) -> Q^T/K^T/V^T in [dh, c, t] layout -> per block:
  PE-transpose V tiles to subtoken-major; per 512-query group:
  E^T = K^T_tile.T @ Q^T (fp32 PSUM), ACT exp -> bf16 SBUF,
  AV^T = Vb_tile.T @ exp^T (fp32 accum), row sums via ones-vector matmul,
  reciprocal + bf16 PE broadcast, normalize -> out-proj lhsT layout
  -> out proj (bf16, bias via K=1 ones matmul) + fp32 residual -> DMA out.
"""

import numpy as np
import ml_dtypes

B, S, D, H = 2, 2048, 1024, 8
DH = D // H          # 128
EPS = 1e-5
NCORES = 8
T = (B * S) // NCORES  # 512 token rows per core
NBLK = 2               # 256-token attention blocks per core
A = 2048               # subtokens per block
QG = 512               # query-group size (= one c-pair x 256 t)
NQG = A // QG          # 4
NKT = A // 128         # 16 k-tiles per block

bf16 = ml_dtypes.bfloat16

_NC_CACHE = {}


def _build_bass(with_bias=True):
    import concourse.bass as bass
    import concourse.mybir as mybir
    import concourse.tile as tile
    from concourse import bacc
    from concourse.masks import make_identity

    f32 = mybir.dt.float32
    bf = mybir.dt.bfloat16
    AF = mybir.ActivationFunctionType
    OP = mybir.AluOpType

    nc = bacc.Bacc()

    x_d = nc.dram_tensor("x", [T, D], f32, kind="ExternalInput")
    wq_d = nc.dram_tensor("wq", [D, D], bf, kind="ExternalInput")
    wk_d = nc.dram_tensor("wk", [D, D], bf, kind="ExternalInput")
    wv_d = nc.dram_tensor("wv", [D, D], bf, kind="ExternalInput")
    wo_d = nc.dram_tensor("wo", [D, D], bf, kind="ExternalInput")
    if with_bias:
        bq_d = nc.dram_tensor("bq", [1, D], bf, kind="ExternalInput")
        bk_d = nc.dram_tensor("bk", [1, D], bf, kind="ExternalInput")
        bv_d = nc.dram_tensor("bv", [1, D], bf, kind="ExternalInput")
        bo_d = nc.dram_tensor("bo", [1, D], bf, kind="ExternalInput")
    out_d = nc.dram_tensor("out", [T, D], f32, kind="ExternalOutput")

    x_r = x_d[:, :].rearrange("(i p) d -> i p d", p=128)      # [4,128,1024]
    out_r = out_d[:, :].rearrange("(i p) d -> i p d", p=128)

    with tile.TileContext(nc) as tc:
        from contextlib import ExitStack
        with ExitStack() as ctx:
            consts = ctx.enter_context(tc.tile_pool(name="consts", bufs=1))
            sb1 = ctx.enter_context(tc.tile_pool(name="sb1", bufs=1))
            work = ctx.enter_context(tc.tile_pool(name="work", bufs=4))
            xh_p = ctx.enter_context(tc.tile_pool(name="xh", bufs=2))
            exp_p = ctx.enter_context(tc.tile_pool(name="expT", bufs=2))
            rec_p = ctx.enter_context(tc.tile_pool(name="rec", bufs=2))
            bc_p = ctx.enter_context(tc.tile_pool(name="bcsb", bufs=2))
            out_p = ctx.enter_context(tc.tile_pool(name="outsb", bufs=3))

            # ---------- DMA inputs ----------
            # One shared HBM pipe services all DMAs, so issue in need-order:
            # x (LN starts immediately), wq, wk, biases, wv, wo. Alternate the
            # two HWDGE rings so queue dispatch isn't the bottleneck.
            x_sb = sb1.tile([128, 4, D], f32, tag="x")
            for i in range(4):
                nc.sync.dma_start(out=x_sb[:, i, :], in_=x_r[i])

            w_sb = {}
            b_sb = {}

            def load_w(name, d):
                w_sb[name] = consts.tile([128, 8, D], bf, tag=name, name=name)
                w_r = d[:, :].rearrange("(c p) n -> c p n", p=128)
                for c in range(8):
                    nc.sync.dma_start(out=w_sb[name][:, c, :], in_=w_r[c])

            bo_sb = None
            if with_bias:
                for name, d in (("bq", bq_d), ("bk", bk_d), ("bv", bv_d)):
                    b_sb[name] = consts.tile([1, D], bf, tag=name, name=name)
                    nc.sync.dma_start(out=b_sb[name], in_=d[:, :])
                bo_sb = consts.tile([1, D], bf, tag="bo")
                nc.sync.dma_start(out=bo_sb, in_=bo_d[:, :])
            load_w("wq", wq_d)
            load_w("wk", wk_d)
            load_w("wv", wv_d)
            load_w("wo", wo_d)
            wo_sb = w_sb["wo"]

            ident = consts.tile([128, 128], bf, tag="ident")
            make_identity(nc, ident)
            ones_col = consts.tile([128, 1], bf, tag="ones_col")
            nc.vector.memset(ones_col, 1.0)
            ones_row_b = consts.tile([1, 128], bf, tag="ones_row_b")
            nc.vector.memset(ones_row_b, 1.0)
            ones_row_w = consts.tile([1, QG], bf, tag="ones_row_w")
            nc.vector.memset(ones_row_w, 1.0)
            eps_sb = consts.tile([128, 1], f32, tag="eps")
            nc.vector.memset(eps_sb, EPS)

            # plain transposed projections: [dh, c, t-global]
            qTp = sb1.tile([128, 8, T], bf, tag="qTp")
            kTp = sb1.tile([128, 8, T], bf, tag="kTp")
            vTp = sb1.tile([128, 8, T], bf, tag="vTp")
            xhT = sb1.tile([128, 8, T], bf, tag="xhT")
            vb = sb1.tile([128, NBLK, NKT, 128], bf, tag="vb")
            # out-proj lhsT layout: [dh, h, c, t-local]
            aT = sb1.tile([128, NBLK, 8, 256], bf, tag="aT")

            # ========== phase A: LN, transposes, projections ==========
            # psB_et outlives phase A; open it first so pool release stays
            # stack-ordered when the phase-A pools close mid-kernel.
            psB_et = ctx.enter_context(
                tc.tile_pool(name="psB_et", bufs=2, space="PSUM"))
            psA = ExitStack()
            psA_mm = psA.enter_context(
                tc.tile_pool(name="psA_mm", bufs=2, space="PSUM"))
            psA_tr = psA.enter_context(
                tc.tile_pool(name="psA_tr", bufs=2, space="PSUM"))
            if True:

                # PE warm-up: keep TensorE busy during LN so the first real
                # matmuls run at full clock (HAM un-throttles after ~3.4us of
                # matmul activity; transpose-mode doesn't count as PE-busy).
                for wu in range(48):
                    wt = psA_mm.tile([128, 128], f32, tag="mm",
                                     name=f"warm{wu}")
                    nc.tensor.matmul(wt, lhsT=ident, rhs=ident,
                                     start=True, stop=True)

                # LN + xhat transpose
                for i in range(4):
                    stats = work.tile([128, 2, 6], f32, tag="stats")
                    for s in range(2):
                        nc.vector.bn_stats(
                            out=stats[:, s, :], in_=x_sb[:, i, s * 512:(s + 1) * 512]
                        )
                    mv = work.tile([128, 2], f32, tag="mv")
                    nc.vector.bn_aggr(out=mv, in_=stats)
                    std = work.tile([128, 1], f32, tag="std")
                    nc.scalar.activation(
                        out=std, in_=mv[:, 1:2], func=AF.Sqrt,
                        bias=eps_sb, scale=1.0,
                    )
                    rstd = work.tile([128, 1], f32, tag="rstd")
                    nc.vector.reciprocal(out=rstd, in_=std)
                    xh_i = xh_p.tile([128, D], bf, tag="xh")
                    nc.vector.tensor_scalar(
                        out=xh_i, in0=x_sb[:, i, :],
                        scalar1=mv[:, 0:1], scalar2=rstd,
                        op0=OP.subtract, op1=OP.mult,
                    )
                    for cb in range(2):
                        trb = psA_tr.tile([128, 4, 128], bf, tag="tr",
                                          name=f"xtr{i}_{cb}")
                        for cc in range(4):
                            c = cb * 4 + cc
                            nc.tensor.transpose(
                                trb[:, cc, :],
                                xh_i[:, c * 128:(c + 1) * 128], ident,
                            )
                        nc.scalar.copy(
                            out=xhT[:, cb * 4:(cb + 1) * 4,
                                    i * 128:(i + 1) * 128],
                            in_=trb,
                        )

                # preload the exp table set while projections run, so the
                # first attention exp doesn't pay the ~2.7us table switch
                extbl = work.tile([128, 1], f32, tag="extbl")
                nc.scalar.activation(out=extbl, in_=eps_sb, func=AF.Exp)

                # QKV projections -> plain [dh, c, t]
                def proj_c(wname, bname, dst, c):
                    ps = psA_mm.tile([128, T], f32, tag="mm",
                                     name=f"mm_{wname}_{c}")
                    for kk in range(8):
                        nc.tensor.matmul(
                            ps,
                            lhsT=w_sb[wname][:, kk, c * 128:(c + 1) * 128],
                            rhs=xhT[:, kk, :],
                            start=(kk == 0),
                            stop=(not with_bias and kk == 7),
                        )
                    # bias via K=1 ones-matmul into the accumulation
                    if with_bias:
                        nc.tensor.matmul(
                            ps, lhsT=b_sb[bname][:, c * 128:(c + 1) * 128],
                            rhs=ones_row_w, start=False, stop=True,
                        )
                    # cast-copy on DVE (idle during projections; keep
                    # ACT clear ahead of the exp stream)
                    nc.vector.tensor_copy(out=dst[:, c, :], in_=ps)


            # ========== phase B: attention + per-block out-projection ==========
            # PSUM while phase A is draining: psA (2+2) + et (2x2) = 8 banks.
            # After psA closes: et 4 + av 1 + sm 1 + o (bcast/out-proj) 2 = 8.
            if True:
                qgs = [(h, g) for h in range(NBLK) for g in range(NQG)]
                state = {}

                def qg_create(idx):
                    h, g = qgs[idx]
                    t0 = h * 256 + g * 64
                    state[idx] = {
                        "q_rhs": qTp[:, :, t0:t0 + 64],
                        "expT": exp_p.tile([128, NKT, QG], bf, tag="expT",
                                           name=f"expT{idx}"),
                    }

                def qk_chunks(idx, lo, hi):
                    h, g = qgs[idx]
                    st = state[idx]
                    for chunk in range(lo, hi):
                        et = psB_et.tile([128, 2, QG], f32, tag="etb",
                                         name=f"et{idx}_{chunk}")
                        for jj in range(2):
                            kt = chunk * 2 + jj
                            c, half = kt // 2, kt % 2
                            nc.tensor.matmul(
                                et[:, jj, :],
                                lhsT=kTp[:, c, h * 256 + half * 128:
                                         h * 256 + half * 128 + 128],
                                rhs=st["q_rhs"],
                                start=True, stop=True,
                            )
                        nc.scalar.activation(
                            out=st["expT"][:, chunk * 2:(chunk + 1) * 2, :],
                            in_=et, func=AF.Exp,
                        )

                def av_part(idx):
                    h, g = qgs[idx]
                    st = state[idx]
                    av = psB_av.tile([128, QG], f32, tag="av", name=f"av{idx}")
                    st["av"] = av
                    for kt in range(NKT):
                        nc.tensor.matmul(
                            av, lhsT=vb[:, h, kt, :], rhs=st["expT"][:, kt, :],
                            start=(kt == 0), stop=(kt == NKT - 1),
                        )

                def sums_part(idx):
                    st = state[idx]
                    sm = psB_sm.tile([1, QG], f32, tag="sm", name=f"sm{idx}")
                    st["sm"] = sm
                    for kt in range(NKT):
                        nc.tensor.matmul(
                            sm, lhsT=ones_col, rhs=st["expT"][:, kt, :],
                            start=(kt == 0), stop=(kt == NKT - 1),
                        )

                def bcast_norm(idx):
                    h, g = qgs[idx]
                    st = state[idx]
                    rec = rec_p.tile([1, QG], f32, tag="rec", name=f"rec{idx}")
                    nc.vector.reciprocal(out=rec, in_=st["sm"])
                    recb = rec_p.tile([1, QG], bf, tag="recb", name=f"recb{idx}")
                    nc.vector.tensor_copy(out=recb, in_=rec)
                    bc_ps = psB_o.tile([128, QG], f32, tag="o", name=f"bcp{idx}")
                    nc.tensor.matmul(
                        bc_ps, lhsT=ones_row_b, rhs=recb, start=True, stop=True
                    )
                    bc_sb = bc_p.tile([128, QG], f32, tag="bcsb", name=f"bcs{idx}")
                    nc.vector.tensor_copy(out=bc_sb, in_=bc_ps)
                    # normalize; q order (c,t) matches aT [h, c, t] layout
                    nc.vector.tensor_mul(
                        out=aT[:, h, :, g * 64:(g + 1) * 64],
                        in0=st["av"].rearrange("p (c t) -> p c t", c=8),
                        in1=bc_sb.rearrange("p (c t) -> p c t", c=8),
                    )

                def outproj_unit(h, it, nh):
                    i = h * 2 + it
                    tl = it * 128
                    nsl = slice(nh * 512, (nh + 1) * 512)
                    ps = psB_o.tile([128, 512], f32, tag="o",
                                    name=f"op{h}_{it}_{nh}")
                    for c in range(8):
                        nc.tensor.matmul(
                            ps,
                            lhsT=aT[:, h, c, tl:tl + 128],
                            rhs=wo_sb[:, c, nsl],
                            start=(c == 0),
                            stop=(not with_bias and c == 7),
                        )
                    if with_bias:
                        nc.tensor.matmul(
                            ps, lhsT=ones_row_b, rhs=bo_sb[:, nsl],
                            start=False, stop=True,
                        )
                    ot = out_p.tile([128, 512], f32, tag="ot",
                                    name=f"ot{h}_{it}_{nh}")
                    nc.vector.tensor_add(
                        out=ot, in0=ps, in1=x_sb[:, i, nsl]
                    )
                    nc.sync.dma_start(out=out_r[i][:, nsl], in_=ot)

                # depth-1 software pipeline over query groups: QK/exp of group
                # i overlaps AV/sums/normalize of group i-1 so ACT (exp) never
                # drains between groups.
                # projections, with attention for qg 0 started as early as
                # possible: QK chunk c only needs kTp column c, so it is
                # emitted right after that column's K projection; V transposes
                # follow each V projection column.
                for c in range(8):
                    proj_c("wq", "bq", qTp, c)
                for c in range(8):
                    proj_c("wk", "bk", kTp, c)
                qg_create(0)
                qk_chunks(0, 0, 2)
                for c in range(8):
                    proj_c("wv", "bv", vTp, c)
                    if c < 6:
                        qk_chunks(0, 2 + c, 3 + c)
                for h in range(NBLK):
                    for grp in range(NKT // 4):
                        trb = psA_tr.tile([128, 4, 128], bf, tag="tr",
                                          name=f"vtr{h}_{grp}")
                        for jj in range(4):
                            j = grp * 4 + jj
                            c, half = j // 2, j % 2
                            nc.tensor.transpose(
                                trb[:, jj, :],
                                vTp[:, c, h * 256 + half * 128:
                                    h * 256 + half * 128 + 128],
                                ident,
                            )
                        nc.vector.tensor_copy(
                            out=vb[:, h, grp * 4:(grp + 1) * 4, :], in_=trb
                        )
                    qk_chunks(0, 6 + h, 7 + h)
                # phase-A PSUM pools give way to the attention aux pools
                psA.close()
                psB_av = ctx.enter_context(
                    tc.tile_pool(name="psB_av", bufs=1, space="PSUM"))
                psB_sm = ctx.enter_context(
                    tc.tile_pool(name="psB_sm", bufs=1, space="PSUM"))
                psB_o = ctx.enter_context(
                    tc.tile_pool(name="psB_o", bufs=2, space="PSUM"))
                NG = len(qgs)
                # out-proj units (9 matmuls each) become ready one pass after
                # the norms they read; spread one unit per pass to keep the
                # PE backlog per exp-window flat.
                op_sched = {2: (0, 0, 0), 3: (0, 0, 1), 4: (0, 1, 0),
                            5: (0, 1, 1), 6: (1, 0, 0), 7: (1, 0, 1)}
                for i in range(1, NG + 1):
                    prev = i - 1
                    if i == NG:
                        sums_part(prev)
                        av_part(prev)
                    else:
                        av_part(prev)
                        qg_create(i)
                        qk_chunks(i, 0, 2)
                        sums_part(prev)
                    if i < NG:
                        qk_chunks(i, 2, 4)
                    bcast_norm(prev)
                    if i in op_sched:
                        outproj_unit(*op_sched[i])
                    if i < NG:
                        qk_chunks(i, 4, 8)
                outproj_unit(1, 1, 0)
                outproj_unit(1, 1, 1)

    nc.compile()
    return nc


def _get_nc(with_bias=True):
    if with_bias not in _NC_CACHE:
        _NC_CACHE[with_bias] = _build_bass(with_bias)
    return _NC_CACHE[with_bias]


def kernel(**inputs):
    from concourse.bass_utils import run_bass_kernel_spmd

    q = np.asarray(inputs["q"], np.float32)
    Wq = np.asarray(inputs["Wq"], np.float32)
    Wk = np.asarray(inputs["Wk"], np.float32)
    Wv = np.asarray(inputs["Wv"], np.float32)
    Wo = np.asarray(inputs["Wo"], np.float32)
    bq = np.asarray(inputs["bq"], np.float32)
    bk = np.asarray(inputs["bk"], np.float32)
    bv = np.asarray(inputs["bv"], np.float32)
    bo = np.asarray(inputs["bo"], np.float32)
    gamma = np.asarray(inputs["gamma"], np.float32)
    beta = np.asarray(inputs["beta"], np.float32)

    # fold LN affine into QKV weights; fold post-softmax 1/sqrt(D) into V
    sc = 1.0 / np.sqrt(np.float32(D))
    wqT = np.ascontiguousarray((gamma[:, None] * Wq.T)).astype(bf16)
    wkT = np.ascontiguousarray((gamma[:, None] * Wk.T)).astype(bf16)
    wvT = np.ascontiguousarray((gamma[:, None] * Wv.T) * sc).astype(bf16)
    woT = np.ascontiguousarray(Wo.T).astype(bf16)
    bq_e = np.ascontiguousarray((beta @ Wq.T + bq).reshape(1, D)).astype(bf16)
    bk_e = np.ascontiguousarray((beta @ Wk.T + bk).reshape(1, D)).astype(bf16)
    bv_e = np.ascontiguousarray(((beta @ Wv.T + bv) * sc).reshape(1, D)).astype(bf16)
    bo_b = np.ascontiguousarray(bo.reshape(1, D)).astype(bf16)

    X = np.ascontiguousarray(q.reshape(B * S, D))
    with_bias = not (
        np.all(bq_e == 0) and np.all(bk_e == 0)
        and np.all(bv_e == 0) and np.all(bo_b == 0)
    )
    base = {"wq": wqT, "wk": wkT, "wv": wvT, "wo": woT}
    if with_bias:
        base.update({"bq": bq_e, "bk": bk_e, "bv": bv_e, "bo": bo_b})
    in_maps = [
        {**base, "x": np.ascontiguousarray(X[T * c:T * (c + 1)])}
        for c in range(NCORES)
    ]

    nc = _get_nc(with_bias)
    res = run_bass_kernel_spmd(nc, in_maps, core_ids=list(range(NCORES)))
    global LAST_RESULT
    LAST_RESULT = res
    out = np.concatenate([res.results[c]["out"] for c in range(NCORES)], axis=0)
    return out.reshape(B, S, D).astype(np.float32)


LAST_RESULT = None
